# revision 2
# baseline (speedup 1.0000x reference)
"""Multi-head attention layer on 8 TRN2 NeuronCores (v3 — DoubleRow scores).

Reference computation (fp32):
    q = query @ Wq + bq; k = key @ Wk + bk; v = value @ Wv + bv
    scores = softmax(q @ k.T / sqrt(64)) per head
    out = (scores @ v) @ Wo + bo

Sharding (tensor-parallel over head halves x data-parallel over batch):
core c = 2*b + hh handles batch b and head-half hh (heads hh*8..hh*8+8).
Host sums the two out-projection partials per batch.

v3 structure (308.8us vs v2's 351.6us; ACT exp is the bottleneck:
256 x [128,1024] exps = 265.7us engine-busy floor at 1 elem/cyc/lane):
  * scores run in fp8e4 DoubleRow perf mode (0.5 cycles/row): qT holds
    (hi, lo) e4m3 planes (hi = e4m3(q+bq), lo = e4m3(q+bq-hi) via one
    ln_bwd_dx), kTt single e4m3; lhsT = k broadcast_to a stride-0
    middle dim, rhs = the two qT planes, so one matmul contracts
    k.(qh+ql) = k.q at half PE rate (scores 109us -> 55us PE busy;
    q/k quantization moves e3m4+e3m4 -> ~exact+e4m3: rel err 1.19e-2
    -> 1.42e-2, still under the 2e-2 gate).
  * deadline/not-before work queue (whole-tile atomic!) paces q/k/v
    projection + lqh0 out-proj matmul pairs between score tiles; v is
    split per (Lk-tile, head-pair) so slot-0 attnv only waits on head
    0 columns.  Deadlines are strictly before first use: the pump runs
    AFTER the score tile at pos, and PE executes in emission order.
  * prologue: q-path DMAs (wq mt0 slice + 2 xq quarters) ship first,
    PE warm-up junk matmuls hold full pstate, hi-plane drains precede
    lo drains, and the first score tile is computed hi-only with split
    512-wide exps -> first exp at ~12us.
  * tail: lqh1 out-proj partials over head-pairs 0-2 (+bo) are staged
    bf16 during slots 14-15 (the x/weights pool closes after slot 13
    to free SBUF); after slot-15 attnv only a single pair-3 matmul +
    DVE tensor_tensor combine + batched 2-tile DMAs remain.
  * outT/staging bf16 (host upcasts + sums partials), biases packed in
    one DMA, ones-columns memset on GPSIMD.
PSUM: scores 2x2-bank ring + proj 2x1-bank ring + attnv 2x1-bank = 8.
Schedule notes: exp(i) reads the 2-slot scores ring; its matmuls (2x
DoubleRow, 107ns each) slip into the previous exp's 1038ns shadow, so
ACT runs ~97% duty mid-stream.  GPSIMD cannot touch PSUM (tail
combines must stay on DVE); PSUM accumulation groups must never
interleave on a bank (hence the whole-tile-atomic pump).
"""

import numpy as np
import ml_dtypes

import concourse.bacc as bacc
import concourse.bass as bass
import concourse.mybir as mybir
import concourse.tile as tile
from concourse import bass_utils

B, L, DIM = 4, 2048, 1024
H, HD = 16, 64
N_CORES = 8
HL = 8             # local heads per core
FD = 512           # local feature columns (8 heads * 64)
KT = DIM // 128    # 8 contraction k-tiles for q/k/v projections
MT = FD // 128     # 4 feature tiles (head pairs)
NLK = L // 128     # 16 Lk tiles
VSTR = 66          # per-head stride in v_sb (64 vals + ones col + pad)

BF16 = mybir.dt.bfloat16
F32 = mybir.dt.float32
FP8 = mybir.dt.float8e3    # e3m4: x inputs
FP8E4 = mybir.dt.float8e4  # e4m3: q/k activations (DoubleRow operands)
AF = mybir.ActivationFunctionType
DR = mybir.MatmulPerfMode.DoubleRow
ADD = mybir.AluOpType.add


def _build_body(tc, io):
    nc = tc.nc
    (xq, xk, xv, wq, wk, wv, wo, biases, bvr, ident, outT) = io

    from contextlib import ExitStack
    with ExitStack() as ctx:
        const = ctx.enter_context(tc.tile_pool(name="const", bufs=1))
        wpool = ctx.enter_context(tc.tile_pool(name="wpool", bufs=1))
        qk_sb = ctx.enter_context(tc.tile_pool(name="qk_sb", bufs=1))
        e_pool = ctx.enter_context(tc.tile_pool(name="e_pool", bufs=24))
        norm_pool = ctx.enter_context(tc.tile_pool(name="norm", bufs=7))
        rec_pool = ctx.enter_context(tc.tile_pool(name="rec", bufs=3))
        stage = ctx.enter_context(tc.tile_pool(name="stage", bufs=4))
        s_ps = ctx.enter_context(
            tc.tile_pool(name="s_ps", bufs=2, space="PSUM"))
        p_ps = ctx.enter_context(
            tc.tile_pool(name="p_ps", bufs=2, space="PSUM"))
        av_ps = ctx.enter_context(
            tc.tile_pool(name="av_ps", bufs=2, space="PSUM"))
        # x inputs + qkv weights live only through slot 13; their pool
        # closes before the out-proj partial staging pool opens.
        xw_ctx = ExitStack()
        xw = xw_ctx.enter_context(tc.tile_pool(name="xw", bufs=1))

        # ---- constants (bq|bqn|bk|bo packed in one DMA) ----
        bias_sb = const.tile([128, 3 * MT + KT], F32)
        bq_sb = bias_sb[:, 0:MT]
        bqn_sb = bias_sb[:, MT:2 * MT]
        bk_sb = bias_sb[:, 2 * MT:3 * MT]
        bo_sb = bias_sb[:, 3 * MT:3 * MT + KT]
        id_sb = const.tile([128, 128], BF16)

        # ---- persistent activations ----
        # qT planes: [:, mt, 0, :] = e4m3(q), [:, mt, 1, :] = e4m3(q - hi)
        qT = qk_sb.tile([128, MT, 2, L], FP8E4)
        kTt = qk_sb.tile([128, MT, L], FP8E4)
        v_sb = qk_sb.tile([128, NLK, HL * VSTR], BF16)
        oT_all = qk_sb.tile([128, MT, L], BF16)

        # ones column of v_aug (proj drains fill the value columns);
        # on GPSIMD so the DVE is free for the prologue drains
        for h in range(HL):
            nc.gpsimd.memset(v_sb[:, :, h * VSTR + 64:h * VSTR + 65], 1.0)
            nc.gpsimd.memset(v_sb[:, :, h * VSTR + 65:h * VSTR + 66], 0.0)

        # ---- weights + x inputs, DMA'd in first-use order ----
        wq_sb = xw.tile([128, KT, FD], BF16, tag="wq")
        wk_sb = xw.tile([128, KT, FD], BF16, tag="wk")
        wv_sb = xw.tile([128, KT, FD], BF16, tag="wv")
        wo_sb = wpool.tile([128, MT, DIM], BF16, tag="wo")
        xq_sb = xw.tile([128, KT, L], FP8, name="xq_sb")
        xk_sb = xw.tile([128, KT, L], FP8, name="xk_sb")
        xv_sb = xw.tile([128, KT, L], BF16, name="xv_sb")

        def dma(dst, src):
            nc.sync.dma_start(out=dst, in_=src)

        h1 = slice(1024, 2048)
        # stream-start critical path: the short k(0, 0:512) projection
        # ships + projects first (overlapping the q DMAs); the q path
        # (wq-mt0 + two xq quarters) is the longer pole.
        dma(wq_sb[:, :, 0:128], wq[:, :, 0:128])
        dma(xq_sb[:, :, 0:512], xq[:, :, 0:512])
        dma(xq_sb[:, :, 512:1024], xq[:, :, 512:1024])
        dma(bias_sb, biases)
        dma(wk_sb[:, :, 0:128], wk[:, :, 0:128])
        dma(xk_sb[:, :, 0:256], xk[:, :, 0:256])
        dma(xk_sb[:, :, 256:512], xk[:, :, 256:512])
        dma(xk_sb[:, :, 512:1024], xk[:, :, 512:1024])
        dma(xk_sb[:, :, h1], xk[:, :, h1])
        # v path
        dma(wv_sb, wv)
        bv_bc = const.tile([128, FD], BF16)
        dma(bv_bc, bass.AP(tensor=bvr.tensor, offset=bvr.offset,
                           ap=[[0, 128], [1, FD]]))
        for quarter in range(4):
            sl = slice(quarter * 512, (quarter + 1) * 512)
            dma(xv_sb[:, :, sl], xv[:, :, sl])
        # weight remainders (first needed at slot 2, pos 32)
        dma(wq_sb[:, :, 128:FD], wq[:, :, 128:FD])
        dma(wk_sb[:, :, 128:FD], wk[:, :, 128:FD])
        # lqh1 q inputs + out-proj weights
        dma(xq_sb[:, :, h1], xq[:, :, h1])
        dma(id_sb, ident)
        dma(wo_sb, wo)

        # ---- work units ----
        def q_proj_mms(ps, mt, c0, kt0, kt1):
            for kt in range(kt0, kt1):
                nc.tensor.matmul(
                    ps, wq_sb[:, kt, mt * 128:(mt + 1) * 128],
                    xq_sb[:, kt, c0:c0 + 512],
                    start=(kt == 0), stop=(kt == KT - 1))

        def q_proj_drain(ps, mt, c0):
            hi = qT[:, mt, 0, c0:c0 + 512]
            nc.vector.tensor_scalar(
                out=hi, in0=ps, scalar1=bq_sb[:, mt:mt + 1],
                scalar2=None, op0=ADD)
            # lo = ps - hi + bq  (ln_bwd_dx: dy - x_hat*s0 - s1)
            nc.vector.ln_bwd_dx(
                out=qT[:, mt, 1, c0:c0 + 512], dy=ps, x_hat=hi,
                mean_dyx=1.0, mean_dy=bqn_sb[:, mt:mt + 1])

        def k_proj_tile(mt, c0, w, pool=None, tag="proj"):
            """kTt cols [c0, c0+w) for feature tile mt (single e4m3)."""
            ps = (pool or p_ps).tile([128, 512], F32, tag=tag,
                                     name=f"k{mt}c{c0}")
            for kt in range(KT):
                nc.tensor.matmul(
                    ps[:, 0:w], wk_sb[:, kt, mt * 128:(mt + 1) * 128],
                    xk_sb[:, kt, c0:c0 + w],
                    start=(kt == 0), stop=(kt == KT - 1))
            nc.vector.tensor_scalar(
                out=kTt[:, mt, c0:c0 + w], in0=ps[:, 0:w],
                scalar1=bk_sb[:, mt:mt + 1], scalar2=None, op0=ADD)

        def v_proj_mms(st, rt, hg, kt0, kt1):
            if kt0 == 0:
                st["ps"] = p_ps.tile([128, 128], F32, tag="proj",
                                     name=f"v{rt}g{hg}")
            ps = st["ps"]
            for kt in range(kt0, kt1):
                nc.tensor.matmul(
                    ps, xv_sb[:, kt, rt * 128:(rt + 1) * 128],
                    wv_sb[:, kt, hg * 128:hg * 128 + 128],
                    start=(kt == 0), stop=(kt == KT - 1))
            if kt1 == KT:
                ps = st.pop("ps")
                f0 = hg * 128
                dst = v_sb[:, rt,
                           2 * hg * VSTR:(2 * hg + 2) * VSTR].rearrange(
                    "p (h d) -> p h d", d=VSTR)[:, :, 0:64]
                nc.vector.tensor_tensor(
                    out=dst, in0=ps.rearrange("p (h d) -> p h d", d=64),
                    in1=bv_bc[:, f0:f0 + 128].rearrange(
                        "p (h d) -> p h d", d=64),
                    op=ADD)

        def oproj_mms(ps, mt, c0, kt0, kt1):
            for kt in range(kt0, kt1):
                nc.tensor.matmul(
                    ps, wo_sb[:, kt, mt * 128:(mt + 1) * 128],
                    oT_all[:, kt, c0:c0 + 512],
                    start=(kt == 0), stop=(kt == MT - 1))

        def oproj_drain(ps, mt, c0):
            st = stage.tile([128, 512], BF16, tag="stage")
            nc.vector.tensor_scalar(
                out=st, in0=ps, scalar1=bo_sb[:, mt:mt + 1],
                scalar2=None, op0=ADD)
            nc.sync.dma_start(
                out=outT[mt * 128:(mt + 1) * 128, c0:c0 + 512], in_=st)

        def oproj_tile(mt, n):
            ps = p_ps.tile([128, 512], F32, tag="proj", name=f"o{mt}n{n}")
            oproj_mms(ps, mt, n * 512, 0, MT)
            oproj_drain(ps, mt, n * 512)

        # ---- atom queue: (deadline_pos, not_before_pos, fn), in priority
        # order.  pos = global score-tile index (s*16 + lkt, 0..255). ----
        atoms = []

        def add_atom(dl, nb, fn):
            atoms.append((dl, nb, fn))

        def add_k_tile(mt, n, dl, nb):
            st = {}
            c0 = n * 512

            def part(i):
                def f():
                    if i == 0:
                        st["ps"] = p_ps.tile([128, 512], F32, tag="proj",
                                             name=f"k{mt}n{n}")
                    for kt in range(2 * i, 2 * i + 2):
                        nc.tensor.matmul(
                            st["ps"], wk_sb[:, kt, mt * 128:(mt + 1) * 128],
                            xk_sb[:, kt, c0:c0 + 512],
                            start=(kt == 0), stop=(kt == KT - 1))
                    if i == 3:
                        ps = st.pop("ps")
                        nc.vector.tensor_scalar(
                            out=kTt[:, mt, c0:c0 + 512], in0=ps,
                            scalar1=bk_sb[:, mt:mt + 1], scalar2=None,
                            op0=ADD)
                return f
            for i in range(4):
                add_atom(dl, nb, part(i))

        def add_q_tile(mt, n, dl, nb):
            st = {}
            c0 = n * 512

            def part(i):
                def f():
                    if i == 0:
                        st["ps"] = p_ps.tile([128, 512], F32, tag="proj",
                                             name=f"q{mt}n{n}")
                    q_proj_mms(st["ps"], mt, c0, 2 * i, 2 * i + 2)
                    if i == 3:
                        q_proj_drain(st.pop("ps"), mt, c0)
                return f
            for i in range(4):
                add_atom(dl, nb, part(i))

        def add_v_tile(rt, hg, dl, nb):
            st = {}
            add_atom(dl, nb,
                     lambda rt=rt, hg=hg, st=st: v_proj_mms(st, rt, hg, 0, 4))
            add_atom(dl, nb,
                     lambda rt=rt, hg=hg, st=st: v_proj_mms(st, rt, hg, 4, 8))

        def add_oproj_tile(mt, n, dl, nb):
            st = {}

            def part(i):
                def f():
                    if i == 0:
                        st["ps"] = p_ps.tile([128, 512], F32, tag="proj",
                                             name=f"o{mt}n{n}")
                    oproj_mms(st["ps"], mt, n * 512, 2 * i, 2 * i + 2)
                    if i == 1:
                        oproj_drain(st.pop("ps"), mt, n * 512)
                return f
            for i in range(2):
                add_atom(dl, nb, part(i))

        # k feature tile 0, cols 256:512 due before score (0,2)
        # (prologue covers cols 0:256)
        # k(0, 256:512): 2-atom queue tile right behind the prologue
        k0b_st = {}

        def k0b_part(kt0, kt1):
            if kt0 == 0:
                k0b_st["ps"] = p_ps.tile([128, 512], F32, tag="proj",
                                         name="k0b")
            ps = k0b_st["ps"]
            for kt in range(kt0, kt1):
                nc.tensor.matmul(
                    ps[:, 0:256], wk_sb[:, kt, 0:128],
                    xk_sb[:, kt, 256:512],
                    start=(kt == 0), stop=(kt == KT - 1))
            if kt1 == KT:
                nc.vector.tensor_scalar(
                    out=kTt[:, 0, 256:512], in0=k0b_st.pop("ps")[:, 0:256],
                    scalar1=bk_sb[:, 0:1], scalar2=None, op0=ADD)
        add_atom(1, 0, lambda: k0b_part(0, 4))
        add_atom(1, 0, lambda: k0b_part(4, 8))
        add_k_tile(0, 1, dl=2, nb=0)
        add_k_tile(0, 2, dl=6, nb=3)
        add_k_tile(0, 3, dl=10, nb=4)
        # v per head-pair hg0: due before slot-1's chunk 0 (pos ~21);
        # xv quarter b lands ~pos 7+3b -> stagger not_before.
        for rt in range(NLK):
            add_v_tile(rt, 0, dl=16 + rt // 4, nb=(9, 12, 15, 17)[rt // 4])
        # k/q for feature tile mt: k quarter n first used at slot-2mt
        # tile 4n; q quarters 0/1 both needed at slot-2mt tile 0.
        for mt in range(1, 4):
            base = 32 * mt
            nb0 = max(24, base - 30)
            for n in range(2):
                add_q_tile(mt, n, dl=base - 4 + n, nb=nb0)
            for n in range(4):
                # k quarter n is first read at pos base + 4n
                add_k_tile(mt, n, dl=base + 4 * n - 3, nb=nb0)
        # remaining v head-pairs, due before slot 2*hg+1 (chunks of
        # slot 2*hg run there)
        # all 16 v(rt,hg) tiles are read by the first chunk of slot
        # 2hg (which runs at slot 2hg+1 tile 0 = pos 32hg+16)
        for hg in range(1, 4):
            for rt in range(NLK):
                add_v_tile(rt, hg, dl=32 * hg - 2 + rt, nb=32 * hg - 24)
        # lqh1 q quarters due before slot 8 + 2*mt
        for mt in range(4):
            base = 128 + 32 * mt
            for n in (2, 3):
                add_q_tile(mt, n, dl=base - 4 + n, nb=base - 30)
        # out-projection for lqh0 (cols 0:1024): oT pairs complete once
        # slot-7's chunks retire (during slot 8, tiles 0..7)
        idx = 0
        for n in (0, 1):
            for mt in range(KT):
                add_oproj_tile(mt, n, dl=150 + 4 * idx, nb=138)
                idx += 1

        # group atoms into tiles (consecutive same-(dl,nb) entries from
        # one add_*_tile call share psum state), then EDF order.  The
        # pump NEVER interleaves two queue tiles: a tile's atoms always
        # finish before the next tile starts, so the proj-psum ring only
        # ever sees complete accumulation groups in order.
        tiles_q = []
        i = 0
        while i < len(atoms):
            j = i + 1
            while (j < len(atoms) and atoms[j][0] == atoms[i][0]
                   and atoms[j][1] == atoms[i][1] and j - i < 4):
                j += 1
            tiles_q.append((atoms[i][0], atoms[i][1],
                            [a[2] for a in atoms[i:j]]))
            i = j
        tiles_q.sort(key=lambda t: t[0])
        tdone = [False] * len(tiles_q)
        tpart = [0] * len(tiles_q)
        tstate = {"open": None, "lo": 0}

        def _part(i):
            tiles_q[i][2][tpart[i]]()
            tpart[i] += 1
            if tpart[i] >= len(tiles_q[i][2]):
                tdone[i] = True
                tstate["open"] = None
                while (tstate["lo"] < len(tiles_q)
                       and tdone[tstate["lo"]]):
                    tstate["lo"] += 1

        def pump(pos, budget):
            # whole-tile atomicity: at most one queue tile is ever open,
            # so the proj-psum ring sees complete groups in order.
            while True:
                i = tstate["open"]
                if i is not None:
                    if tiles_q[i][0] <= pos:
                        _part(i)
                        continue
                    if budget > 0:
                        budget -= 1
                        _part(i)
                        continue
                    return
                pick = None
                for j in range(tstate["lo"], len(tiles_q)):
                    if not tdone[j] and tpart[j] == 0                             and tiles_q[j][0] <= pos:
                        pick = j
                        break
                if pick is None and budget > 0:
                    for j in range(tstate["lo"], len(tiles_q)):
                        if not tdone[j] and tpart[j] == 0                                 and tiles_q[j][1] <= pos:
                            pick = j
                            budget -= 1
                            break
                if pick is None:
                    return
                tstate["open"] = pick
                _part(pick)

        # ---- prologue: the q path is the long pole (DMA + 16 mms + 4
        # drains); its hi drains come first so the hi-only first score
        # tile fires earliest.  The short k(0, 0:256) path hides in the
        # q path's shadow; k(0, 256:512) goes through the queue. ----
        # PE warm-up: junk matmuls keep the tensor engine continuously
        # busy so the real projections run at full pstate (2.4GHz)
        junk = stage.tile([128, 512], BF16, tag="stage", name="junk")
        nc.vector.memset(junk, 1.0)
        for i in range(14):
            fps = s_ps.tile([128, 1024], F32, tag="big", name=f"fill{i}")
            nc.tensor.matmul(fps[:, 0:512], junk[0:64, 0:128],
                             junk[0:64, :], start=True, stop=True)

        ps00 = p_ps.tile([128, 512], F32, tag="proj", name="pq00")
        q_proj_mms(ps00, 0, 0, 0, KT)
        ps01 = p_ps.tile([128, 512], F32, tag="proj", name="pq01")
        q_proj_mms(ps01, 0, 512, 0, KT)
        nc.vector.tensor_scalar(
            out=qT[:, 0, 0, 0:512], in0=ps00, scalar1=bq_sb[:, 0:1],
            scalar2=None, op0=ADD)
        nc.vector.tensor_scalar(
            out=qT[:, 0, 0, 512:1024], in0=ps01, scalar1=bq_sb[:, 0:1],
            scalar2=None, op0=ADD)
        k_proj_tile(0, 0, 256, pool=s_ps, tag="big")
        nc.vector.ln_bwd_dx(
            out=qT[:, 0, 1, 0:512], dy=ps00, x_hat=qT[:, 0, 0, 0:512],
            mean_dyx=1.0, mean_dy=bqn_sb[:, 0:1])
        nc.vector.ln_bwd_dx(
            out=qT[:, 0, 1, 512:1024], dy=ps01,
            x_hat=qT[:, 0, 0, 512:1024],
            mean_dyx=1.0, mean_dy=bqn_sb[:, 0:1])

        # ---- attention stream ----
        def scores_tile(h, lqh, lkt, split=False):
            ht, hp = h // 2, (h % 2) * 64
            ps = s_ps.tile([128, 1024], F32, tag="big",
                           name=f"s{h}_{lqh}_{lkt}")
            kap = kTt[hp:hp + 64, ht, lkt * 128:(lkt + 1) * 128]
            kap = kap.unsqueeze(1).broadcast_to([64, 2, 128])
            for n2 in range(2):
                c0 = lqh * 1024 + n2 * 512
                nc.tensor.matmul(
                    ps[:, n2 * 512:(n2 + 1) * 512],
                    kap, qT[hp:hp + 64, ht, :, c0:c0 + 512],
                    start=True, stop=True, perf_mode=DR)
            et = e_pool.tile([128, 1024], BF16, tag="e",
                             name=f"e{h}_{lqh}_{lkt}")
            if split:
                # halves fire as soon as each q quarter's drain lands
                nc.scalar.activation(et[:, 0:512], ps[:, 0:512],
                                     AF.Exp, scale=0.125)
                nc.scalar.activation(et[:, 512:1024], ps[:, 512:1024],
                                     AF.Exp, scale=0.125)
            else:
                nc.scalar.activation(et, ps, AF.Exp, scale=0.125)
            return et

        def scores_tile_first():
            # tile (h0, lqh0, lkt0) from the hi plane only (regular
            # matmul): skips the lo-plane drain dependency so the first
            # exps fire ~2us earlier; the lo term is ~2% of q on 1/512
            # of the scores -> negligible.
            ps = s_ps.tile([128, 1024], F32, tag="big", name="s_first")
            et = e_pool.tile([128, 1024], BF16, tag="e", name="e_first")
            kap = kTt[0:64, 0, 0:128]
            for n2 in range(2):
                nc.tensor.matmul(
                    ps[:, n2 * 512:(n2 + 1) * 512], kap,
                    qT[0:64, 0, 0, n2 * 512:(n2 + 1) * 512],
                    start=True, stop=True)
                nc.scalar.activation(
                    et[:, n2 * 512:(n2 + 1) * 512],
                    ps[:, n2 * 512:(n2 + 1) * 512], AF.Exp, scale=0.125)
            return et

        norm_map = {}

        def attnv_chunk(h, lqh, c, e_tiles, alt=False):
            ht, g = h // 2, h % 2
            pool = s_ps if alt else av_ps
            o_t = pool.tile([128, 66], F32, tag="big" if alt else "av",
                            name=f"av{h}_{lqh}_{c}")
            for i, et in enumerate(e_tiles):
                nc.tensor.matmul(
                    o_t, et[:, c * 128:(c + 1) * 128],
                    v_sb[:, i, h * VSTR:h * VSTR + 66],
                    start=(i == 0), stop=(i == NLK - 1))
            rec = rec_pool.tile([128, 1], F32, tag="rec")
            nc.vector.reciprocal(out=rec, in_=o_t[:, 64:65])
            if g == 0:
                nst = norm_pool.tile([128, 128], BF16, tag="norm",
                                     name=f"n{ht}_{lqh}_{c}")
                norm_map[(lqh, ht, c)] = nst
            else:
                nst = norm_map.pop((lqh, ht, c))
            nc.vector.tensor_scalar(
                out=nst[:, g * 64:(g + 1) * 64], in0=o_t[:, 0:64],
                scalar1=rec, scalar2=None, op0=mybir.AluOpType.mult)
            if g == 1:
                c0 = lqh * 1024 + c * 128
                if lqh == 1 and ht == MT - 1:
                    # tail pair: PE transpose avoids DMA dispatch latency
                    tp = av_ps.tile([128, 128], BF16, tag="av",
                                    name=f"tp{c}")
                    for gg in range(2):
                        nc.tensor.matmul(
                            tp[64 * gg:64 * (gg + 1), :],
                            nst[:, gg * 64:(gg + 1) * 64], id_sb,
                            is_transpose=True, start=True, stop=True)
                    nc.vector.tensor_copy(
                        out=oT_all[:, ht, c0:c0 + 128], in_=tp)
                else:
                    nc.sync.dma_start_transpose(
                        out=oT_all[:, ht, c0:c0 + 128], in_=nst)

        e_saved = {}
        for s in range(14):
            lqh, h = s // 8, s % 8
            pl, ph = (s - 1) // 8, (s - 1) % 8
            for t in range(16):
                pos = s * 16 + t
                et = (scores_tile_first() if pos == 0 else
                      scores_tile(h, lqh, t))
                e_saved.setdefault(s, []).append(et)
                if s == 1:
                    # v(hg0) lands ~pos 19: chunks doubled on tiles 5..8
                    # so the e-pool WAR on slot-1 tiles 8+ clears fast
                    in_chunks = 5 <= t < 9
                    if in_chunks:
                        attnv_chunk(ph, pl, 2 * (t - 5), e_saved[0])
                        attnv_chunk(ph, pl, 2 * (t - 5) + 1, e_saved[0])
                else:
                    in_chunks = s >= 1 and t < 8
                    if in_chunks:
                        attnv_chunk(ph, pl, t, e_saved[s - 1])
                pump(pos, 1 if in_chunks else 2)
            if s >= 1:
                e_saved.pop(s - 1)

        # all queue atoms (they reference x/w tiles) must be emitted
        # before the xw pool closes
        pump(10 ** 9, 10 ** 9)
        xw_ctx.close()

        # out-projection partial staging (pairs 0..2 + bo), bf16
        opart = ctx.enter_context(tc.tile_pool(name="opart", bufs=16))
        bstage_pool = ctx.enter_context(tc.tile_pool(name="bstage", bufs=1))
        bstage = bstage_pool.tile([128, 2, KT, 512], BF16)
        part_sb = {}

        def opart_a(mt, n):
            c0 = n * 512
            ps = p_ps.tile([128, 512], F32, tag="proj", name=f"op{mt}n{n}")
            part_sb[(mt, n, "ps")] = ps
            for kt in (0, 1):
                nc.tensor.matmul(
                    ps, wo_sb[:, kt, mt * 128:(mt + 1) * 128],
                    oT_all[:, kt, c0:c0 + 512],
                    start=(kt == 0), stop=False)

        def opart_b(mt, n):
            c0 = n * 512
            ps = part_sb.pop((mt, n, "ps"))
            nc.tensor.matmul(
                ps, wo_sb[:, 2, mt * 128:(mt + 1) * 128],
                oT_all[:, 2, c0:c0 + 512], start=False, stop=True)
            pt = opart.tile([128, 512], BF16, tag="op",
                            name=f"opart{mt}n{n}")
            nc.vector.tensor_scalar(
                out=pt, in0=ps, scalar1=bo_sb[:, mt:mt + 1],
                scalar2=None, op0=ADD)
            part_sb[(mt, n)] = pt

        pq = []
        for n in (2, 3):
            for mt in range(KT):
                pq.append(lambda mt=mt, n=n: opart_a(mt, n))
                pq.append(lambda mt=mt, n=n: opart_b(mt, n))
        pi = [0]

        def ppump(k):
            while k > 0 and pi[0] < len(pq):
                pq[pi[0]]()
                pi[0] += 1
                k -= 1

        # slots 14-15: lqh1 pair 3; out-proj partials pace in the gaps
        for s in (14, 15):
            lqh, h = s // 8, s % 8
            pl, ph = (s - 1) // 8, (s - 1) % 8
            for t in range(16):
                et = scores_tile(h, lqh, t)
                e_saved.setdefault(s, []).append(et)
                if t < 8:
                    attnv_chunk(ph, pl, t, e_saved[s - 1])
                    if s == 15:
                        ppump(1)
                else:
                    ppump(2)
            e_saved.pop(s - 1)

        # ---- tail: slot-15 attnv + pair-3 out-proj contribution ----
        ppump(100)
        lqh, h = 1, 7
        e_tiles = e_saved.pop(15)

        fin_idx = [0]

        def oproj_final(mt, n):
            c0 = n * 512
            # scores ring is idle in the tail: alternate pools so four
            # final tiles are in flight at once
            if fin_idx[0] % 2 == 0:
                ps = p_ps.tile([128, 512], F32, tag="proj",
                               name=f"of{mt}n{n}")
            else:
                ps = s_ps.tile([128, 512], F32, tag="big",
                               name=f"of{mt}n{n}")
            nc.tensor.matmul(
                ps, wo_sb[:, 3, mt * 128:(mt + 1) * 128],
                oT_all[:, 3, c0:c0 + 512], start=True, stop=True)
            # combines on DVE (GPSIMD cannot read PSUM); results land in
            # a batched staging tile (2 tiles per DMA)
            fin_idx[0] += 1
            nc.vector.tensor_tensor(
                out=bstage[:, n - 2, mt, :], in0=ps,
                in1=part_sb.pop((mt, n)), op=ADD)
            if mt % 2 == 1:
                # dst iterates (p, j, c) to match the SBUF source's
                # partition-major order
                dst = outT[(mt - 1) * 128:(mt + 1) * 128,
                           c0:c0 + 512].rearrange("(j p) c -> p j c", p=128)
                nc.sync.dma_start(
                    out=dst, in_=bstage[:, n - 2, mt - 1:mt + 1, :])

        for c in range(8):
            attnv_chunk(h, lqh, c, e_tiles, alt=(c % 2 == 1))
            if c >= 4:
                oproj_final(2 * (c - 4), 2)
                oproj_final(2 * (c - 4) + 1, 2)
        for mt in range(KT):
            oproj_final(mt, 3)


# revision 3
# speedup vs baseline: 1.0009x; 1.0009x over previous
"""Multi-head attention layer on 8 TRN2 NeuronCores (v3 — DoubleRow scores).

Reference computation (fp32):
    q = query @ Wq + bq; k = key @ Wk + bk; v = value @ Wv + bv
    scores = softmax(q @ k.T / sqrt(64)) per head
    out = (scores @ v) @ Wo + bo

Sharding (tensor-parallel over head halves x data-parallel over batch):
core c = 2*b + hh handles batch b and head-half hh (heads hh*8..hh*8+8).
Host sums the two out-projection partials per batch.

v3 structure (308.8us vs v2's 351.6us; ACT exp is the bottleneck:
256 x [128,1024] exps = 265.7us engine-busy floor at 1 elem/cyc/lane):
  * scores run in fp8e4 DoubleRow perf mode (0.5 cycles/row): qT holds
    (hi, lo) e4m3 planes (hi = e4m3(q+bq), lo = e4m3(q+bq-hi) via one
    ln_bwd_dx), kTt single e4m3; lhsT = k broadcast_to a stride-0
    middle dim, rhs = the two qT planes, so one matmul contracts
    k.(qh+ql) = k.q at half PE rate (scores 109us -> 55us PE busy;
    q/k quantization moves e3m4+e3m4 -> ~exact+e4m3: rel err 1.19e-2
    -> 1.42e-2, still under the 2e-2 gate).
  * deadline/not-before work queue (whole-tile atomic!) paces q/k/v
    projection + lqh0 out-proj matmul pairs between score tiles; v is
    split per (Lk-tile, head-pair) so slot-0 attnv only waits on head
    0 columns.  Deadlines are strictly before first use: the pump runs
    AFTER the score tile at pos, and PE executes in emission order.
  * prologue: q-path DMAs (wq mt0 slice + 2 xq quarters) ship first,
    PE warm-up junk matmuls hold full pstate, hi-plane drains precede
    lo drains, and the first score tile is computed hi-only with split
    512-wide exps -> first exp at ~12us.
  * tail: lqh1 out-proj partials over head-pairs 0-2 (+bo) are staged
    bf16 during slots 14-15 (the x/weights pool closes after slot 13
    to free SBUF); after slot-15 attnv only a single pair-3 matmul +
    DVE tensor_tensor combine + batched 2-tile DMAs remain.
  * outT/staging bf16 (host upcasts + sums partials), biases packed in
    one DMA, ones-columns memset on GPSIMD.
PSUM: scores 2x2-bank ring + proj 2x1-bank ring + attnv 2x1-bank = 8.
Schedule notes: exp(i) reads the 2-slot scores ring; its matmuls (2x
DoubleRow, 107ns each) slip into the previous exp's 1038ns shadow, so
ACT runs ~97% duty mid-stream.  GPSIMD cannot touch PSUM (tail
combines must stay on DVE); PSUM accumulation groups must never
interleave on a bank (hence the whole-tile-atomic pump).
"""

import numpy as np
import ml_dtypes

import concourse.bacc as bacc
import concourse.bass as bass
import concourse.mybir as mybir
import concourse.tile as tile
from concourse import bass_utils

B, L, DIM = 4, 2048, 1024
H, HD = 16, 64
N_CORES = 8
HL = 8             # local heads per core
FD = 512           # local feature columns (8 heads * 64)
KT = DIM // 128    # 8 contraction k-tiles for q/k/v projections
MT = FD // 128     # 4 feature tiles (head pairs)
NLK = L // 128     # 16 Lk tiles
VSTR = 66          # per-head stride in v_sb (64 vals + ones col + pad)

BF16 = mybir.dt.bfloat16
F32 = mybir.dt.float32
FP8 = mybir.dt.float8e3    # e3m4: x inputs
FP8E4 = mybir.dt.float8e4  # e4m3: q/k activations (DoubleRow operands)
AF = mybir.ActivationFunctionType
DR = mybir.MatmulPerfMode.DoubleRow
ADD = mybir.AluOpType.add


def _build_body(tc, io):
    nc = tc.nc
    (xq, xk, xv, wq, wk, wv, wo, biases, bvr, ident, outT) = io

    from contextlib import ExitStack
    with ExitStack() as ctx:
        const = ctx.enter_context(tc.tile_pool(name="const", bufs=1))
        wpool = ctx.enter_context(tc.tile_pool(name="wpool", bufs=1))
        qk_sb = ctx.enter_context(tc.tile_pool(name="qk_sb", bufs=1))
        e_pool = ctx.enter_context(tc.tile_pool(name="e_pool", bufs=24))
        norm_pool = ctx.enter_context(tc.tile_pool(name="norm", bufs=7))
        rec_pool = ctx.enter_context(tc.tile_pool(name="rec", bufs=3))
        stage = ctx.enter_context(tc.tile_pool(name="stage", bufs=4))
        s_ps = ctx.enter_context(
            tc.tile_pool(name="s_ps", bufs=2, space="PSUM"))
        p_ps = ctx.enter_context(
            tc.tile_pool(name="p_ps", bufs=2, space="PSUM"))
        av_ps = ctx.enter_context(
            tc.tile_pool(name="av_ps", bufs=2, space="PSUM"))
        # x inputs + qkv weights live only through slot 13; their pool
        # closes before the out-proj partial staging pool opens.
        xw_ctx = ExitStack()
        xw = xw_ctx.enter_context(tc.tile_pool(name="xw", bufs=1))

        # ---- constants (bq|bqn|bk|bo packed in one DMA) ----
        bias_sb = const.tile([128, 3 * MT + KT], F32)
        bq_sb = bias_sb[:, 0:MT]
        bqn_sb = bias_sb[:, MT:2 * MT]
        bk_sb = bias_sb[:, 2 * MT:3 * MT]
        bo_sb = bias_sb[:, 3 * MT:3 * MT + KT]
        id_sb = const.tile([128, 128], BF16)

        # ---- persistent activations ----
        # qT planes: [:, mt, 0, :] = e4m3(q), [:, mt, 1, :] = e4m3(q - hi)
        qT = qk_sb.tile([128, MT, 2, L], FP8E4)
        kTt = qk_sb.tile([128, MT, L], FP8E4)
        v_sb = qk_sb.tile([128, NLK, HL * VSTR], BF16)
        oT_all = qk_sb.tile([128, MT, L], BF16)

        # ones column of v_aug (proj drains fill the value columns);
        # on GPSIMD so the DVE is free for the prologue drains
        for h in range(HL):
            nc.gpsimd.memset(v_sb[:, :, h * VSTR + 64:h * VSTR + 65], 1.0)
            nc.gpsimd.memset(v_sb[:, :, h * VSTR + 65:h * VSTR + 66], 0.0)

        # ---- weights + x inputs, DMA'd in first-use order ----
        wq_sb = xw.tile([128, KT, FD], BF16, tag="wq")
        wk_sb = xw.tile([128, KT, FD], BF16, tag="wk")
        wv_sb = xw.tile([128, KT, FD], BF16, tag="wv")
        wo_sb = wpool.tile([128, MT, DIM], BF16, tag="wo")
        xq_sb = xw.tile([128, KT, L], FP8, name="xq_sb")
        xk_sb = xw.tile([128, KT, L], FP8, name="xk_sb")
        xv_sb = xw.tile([128, KT, L], BF16, name="xv_sb")

        def dma(dst, src):
            nc.sync.dma_start(out=dst, in_=src)

        h1 = slice(1024, 2048)
        # stream-start critical path: the short k(0, 0:512) projection
        # ships + projects first (overlapping the q DMAs); the q path
        # (wq-mt0 + two xq quarters) is the longer pole.
        dma(wq_sb[:, :, 0:128], wq[:, :, 0:128])
        dma(xq_sb[:, :, 0:512], xq[:, :, 0:512])
        dma(xq_sb[:, :, 512:1024], xq[:, :, 512:1024])
        dma(bias_sb, biases)
        dma(wk_sb[:, :, 0:128], wk[:, :, 0:128])
        dma(xk_sb[:, :, 0:256], xk[:, :, 0:256])
        dma(xk_sb[:, :, 256:512], xk[:, :, 256:512])
        dma(xk_sb[:, :, 512:1024], xk[:, :, 512:1024])
        dma(xk_sb[:, :, h1], xk[:, :, h1])
        # v path
        dma(wv_sb, wv)
        bv_bc = const.tile([128, FD], BF16)
        dma(bv_bc, bass.AP(tensor=bvr.tensor, offset=bvr.offset,
                           ap=[[0, 128], [1, FD]]))
        for quarter in range(4):
            sl = slice(quarter * 512, (quarter + 1) * 512)
            dma(xv_sb[:, :, sl], xv[:, :, sl])
        # weight remainders (first needed at slot 2, pos 32)
        dma(wq_sb[:, :, 128:FD], wq[:, :, 128:FD])
        dma(wk_sb[:, :, 128:FD], wk[:, :, 128:FD])
        # lqh1 q inputs + out-proj weights
        dma(xq_sb[:, :, h1], xq[:, :, h1])
        dma(id_sb, ident)
        dma(wo_sb, wo)

        # ---- work units ----
        def q_proj_mms(ps, mt, c0, kt0, kt1):
            for kt in range(kt0, kt1):
                nc.tensor.matmul(
                    ps, wq_sb[:, kt, mt * 128:(mt + 1) * 128],
                    xq_sb[:, kt, c0:c0 + 512],
                    start=(kt == 0), stop=(kt == KT - 1))

        def q_proj_drain(ps, mt, c0):
            hi = qT[:, mt, 0, c0:c0 + 512]
            nc.vector.tensor_scalar(
                out=hi, in0=ps, scalar1=bq_sb[:, mt:mt + 1],
                scalar2=None, op0=ADD)
            # lo = ps - hi + bq  (ln_bwd_dx: dy - x_hat*s0 - s1)
            nc.vector.ln_bwd_dx(
                out=qT[:, mt, 1, c0:c0 + 512], dy=ps, x_hat=hi,
                mean_dyx=1.0, mean_dy=bqn_sb[:, mt:mt + 1])

        def k_proj_tile(mt, c0, w, pool=None, tag="proj"):
            """kTt cols [c0, c0+w) for feature tile mt (single e4m3)."""
            ps = (pool or p_ps).tile([128, 512], F32, tag=tag,
                                     name=f"k{mt}c{c0}")
            for kt in range(KT):
                nc.tensor.matmul(
                    ps[:, 0:w], wk_sb[:, kt, mt * 128:(mt + 1) * 128],
                    xk_sb[:, kt, c0:c0 + w],
                    start=(kt == 0), stop=(kt == KT - 1))
            nc.vector.tensor_scalar(
                out=kTt[:, mt, c0:c0 + w], in0=ps[:, 0:w],
                scalar1=bk_sb[:, mt:mt + 1], scalar2=None, op0=ADD)

        def v_proj_mms(st, rt, hg, kt0, kt1):
            if kt0 == 0:
                st["ps"] = p_ps.tile([128, 128], F32, tag="proj",
                                     name=f"v{rt}g{hg}")
            ps = st["ps"]
            for kt in range(kt0, kt1):
                nc.tensor.matmul(
                    ps, xv_sb[:, kt, rt * 128:(rt + 1) * 128],
                    wv_sb[:, kt, hg * 128:hg * 128 + 128],
                    start=(kt == 0), stop=(kt == KT - 1))
            if kt1 == KT:
                ps = st.pop("ps")
                f0 = hg * 128
                dst = v_sb[:, rt,
                           2 * hg * VSTR:(2 * hg + 2) * VSTR].rearrange(
                    "p (h d) -> p h d", d=VSTR)[:, :, 0:64]
                nc.vector.tensor_tensor(
                    out=dst, in0=ps.rearrange("p (h d) -> p h d", d=64),
                    in1=bv_bc[:, f0:f0 + 128].rearrange(
                        "p (h d) -> p h d", d=64),
                    op=ADD)

        def oproj_mms(ps, mt, c0, kt0, kt1):
            for kt in range(kt0, kt1):
                nc.tensor.matmul(
                    ps, wo_sb[:, kt, mt * 128:(mt + 1) * 128],
                    oT_all[:, kt, c0:c0 + 512],
                    start=(kt == 0), stop=(kt == MT - 1))

        def oproj_drain(ps, mt, c0):
            st = stage.tile([128, 512], BF16, tag="stage")
            nc.vector.tensor_scalar(
                out=st, in0=ps, scalar1=bo_sb[:, mt:mt + 1],
                scalar2=None, op0=ADD)
            nc.sync.dma_start(
                out=outT[mt * 128:(mt + 1) * 128, c0:c0 + 512], in_=st)

        def oproj_tile(mt, n):
            ps = p_ps.tile([128, 512], F32, tag="proj", name=f"o{mt}n{n}")
            oproj_mms(ps, mt, n * 512, 0, MT)
            oproj_drain(ps, mt, n * 512)

        # ---- atom queue: (deadline_pos, not_before_pos, fn), in priority
        # order.  pos = global score-tile index (s*16 + lkt, 0..255). ----
        atoms = []

        def add_atom(dl, nb, fn):
            atoms.append((dl, nb, fn))

        def add_k_tile(mt, n, dl, nb):
            st = {}
            c0 = n * 512

            def part(i):
                def f():
                    if i == 0:
                        st["ps"] = p_ps.tile([128, 512], F32, tag="proj",
                                             name=f"k{mt}n{n}")
                    for kt in range(2 * i, 2 * i + 2):
                        nc.tensor.matmul(
                            st["ps"], wk_sb[:, kt, mt * 128:(mt + 1) * 128],
                            xk_sb[:, kt, c0:c0 + 512],
                            start=(kt == 0), stop=(kt == KT - 1))
                    if i == 3:
                        ps = st.pop("ps")
                        nc.vector.tensor_scalar(
                            out=kTt[:, mt, c0:c0 + 512], in0=ps,
                            scalar1=bk_sb[:, mt:mt + 1], scalar2=None,
                            op0=ADD)
                return f
            for i in range(4):
                add_atom(dl, nb, part(i))

        def add_q_tile(mt, n, dl, nb):
            st = {}
            c0 = n * 512

            def part(i):
                def f():
                    if i == 0:
                        st["ps"] = p_ps.tile([128, 512], F32, tag="proj",
                                             name=f"q{mt}n{n}")
                    q_proj_mms(st["ps"], mt, c0, 2 * i, 2 * i + 2)
                    if i == 3:
                        q_proj_drain(st.pop("ps"), mt, c0)
                return f
            for i in range(4):
                add_atom(dl, nb, part(i))

        def add_v_tile(rt, hg, dl, nb):
            st = {}
            add_atom(dl, nb,
                     lambda rt=rt, hg=hg, st=st: v_proj_mms(st, rt, hg, 0, 4))
            add_atom(dl, nb,
                     lambda rt=rt, hg=hg, st=st: v_proj_mms(st, rt, hg, 4, 8))

        def add_oproj_tile(mt, n, dl, nb):
            st = {}

            def part(i):
                def f():
                    if i == 0:
                        st["ps"] = p_ps.tile([128, 512], F32, tag="proj",
                                             name=f"o{mt}n{n}")
                    oproj_mms(st["ps"], mt, n * 512, 2 * i, 2 * i + 2)
                    if i == 1:
                        oproj_drain(st.pop("ps"), mt, n * 512)
                return f
            for i in range(2):
                add_atom(dl, nb, part(i))

        # k feature tile 0, cols 256:512 due before score (0,2)
        # (prologue covers cols 0:256)
        # k(0, 256:512): 2-atom queue tile right behind the prologue
        k0b_st = {}

        def k0b_part(kt0, kt1):
            if kt0 == 0:
                k0b_st["ps"] = p_ps.tile([128, 512], F32, tag="proj",
                                         name="k0b")
            ps = k0b_st["ps"]
            for kt in range(kt0, kt1):
                nc.tensor.matmul(
                    ps[:, 0:256], wk_sb[:, kt, 0:128],
                    xk_sb[:, kt, 256:512],
                    start=(kt == 0), stop=(kt == KT - 1))
            if kt1 == KT:
                nc.vector.tensor_scalar(
                    out=kTt[:, 0, 256:512], in0=k0b_st.pop("ps")[:, 0:256],
                    scalar1=bk_sb[:, 0:1], scalar2=None, op0=ADD)
        add_atom(1, 0, lambda: k0b_part(0, 4))
        add_atom(1, 0, lambda: k0b_part(4, 8))
        add_k_tile(0, 1, dl=2, nb=0)
        add_k_tile(0, 2, dl=6, nb=3)
        add_k_tile(0, 3, dl=10, nb=4)
        # v per head-pair hg0: due before slot-1's chunk 0 (pos ~21);
        # xv quarter b lands ~pos 7+3b -> stagger not_before.
        for rt in range(NLK):
            add_v_tile(rt, 0, dl=16 + rt // 4, nb=(9, 12, 15, 17)[rt // 4])
        # k/q for feature tile mt: k quarter n first used at slot-2mt
        # tile 4n; q quarters 0/1 both needed at slot-2mt tile 0.
        for mt in range(1, 4):
            base = 32 * mt
            nb0 = max(24, base - 30)
            for n in range(2):
                add_q_tile(mt, n, dl=base - 4 + n, nb=nb0)
            for n in range(4):
                # k quarter n is first read at pos base + 4n
                add_k_tile(mt, n, dl=base + 4 * n - 3, nb=nb0)
        # remaining v head-pairs, due before slot 2*hg+1 (chunks of
        # slot 2*hg run there)
        # all 16 v(rt,hg) tiles are read by the first chunk of slot
        # 2hg (which runs at slot 2hg+1 tile 0 = pos 32hg+16)
        for hg in range(1, 4):
            for rt in range(NLK):
                add_v_tile(rt, hg, dl=32 * hg - 2 + rt, nb=32 * hg - 24)
        # lqh1 q quarters due before slot 8 + 2*mt
        for mt in range(4):
            base = 128 + 32 * mt
            for n in (2, 3):
                add_q_tile(mt, n, dl=base - 4 + n, nb=base - 30)
        # out-projection for lqh0 (cols 0:1024): oT pairs complete once
        # slot-7's chunks retire (during slot 8, tiles 0..7)
        idx = 0
        for n in (0, 1):
            for mt in range(KT):
                add_oproj_tile(mt, n, dl=150 + 4 * idx, nb=138)
                idx += 1

        # group atoms into tiles (consecutive same-(dl,nb) entries from
        # one add_*_tile call share psum state), then EDF order.  The
        # pump NEVER interleaves two queue tiles: a tile's atoms always
        # finish before the next tile starts, so the proj-psum ring only
        # ever sees complete accumulation groups in order.
        tiles_q = []
        i = 0
        while i < len(atoms):
            j = i + 1
            while (j < len(atoms) and atoms[j][0] == atoms[i][0]
                   and atoms[j][1] == atoms[i][1] and j - i < 4):
                j += 1
            tiles_q.append((atoms[i][0], atoms[i][1],
                            [a[2] for a in atoms[i:j]]))
            i = j
        tiles_q.sort(key=lambda t: t[0])
        tdone = [False] * len(tiles_q)
        tpart = [0] * len(tiles_q)
        tstate = {"open": None, "lo": 0}

        def _part(i):
            tiles_q[i][2][tpart[i]]()
            tpart[i] += 1
            if tpart[i] >= len(tiles_q[i][2]):
                tdone[i] = True
                tstate["open"] = None
                while (tstate["lo"] < len(tiles_q)
                       and tdone[tstate["lo"]]):
                    tstate["lo"] += 1

        def pump(pos, budget):
            # whole-tile atomicity: at most one queue tile is ever open,
            # so the proj-psum ring sees complete groups in order.
            while True:
                i = tstate["open"]
                if i is not None:
                    if tiles_q[i][0] <= pos:
                        _part(i)
                        continue
                    if budget > 0:
                        budget -= 1
                        _part(i)
                        continue
                    return
                pick = None
                for j in range(tstate["lo"], len(tiles_q)):
                    if not tdone[j] and tpart[j] == 0                             and tiles_q[j][0] <= pos:
                        pick = j
                        break
                if pick is None and budget > 0:
                    for j in range(tstate["lo"], len(tiles_q)):
                        if not tdone[j] and tpart[j] == 0                                 and tiles_q[j][1] <= pos:
                            pick = j
                            budget -= 1
                            break
                if pick is None:
                    return
                tstate["open"] = pick
                _part(pick)

        # ---- prologue: the q path is the long pole (DMA + 16 mms + 4
        # drains); its hi drains come first so the hi-only first score
        # tile fires earliest.  The short k(0, 0:256) path hides in the
        # q path's shadow; k(0, 256:512) goes through the queue. ----
        # PE warm-up: junk matmuls keep the tensor engine continuously
        # busy so the real projections run at full pstate (2.4GHz)
        junk = stage.tile([128, 512], BF16, tag="stage", name="junk")
        nc.vector.memset(junk, 1.0)
        for i in range(14):
            fps = s_ps.tile([128, 1024], F32, tag="big", name=f"fill{i}")
            nc.tensor.matmul(fps[:, 0:512], junk[0:64, 0:128],
                             junk[0:64, :], start=True, stop=True)

        ps00 = p_ps.tile([128, 512], F32, tag="proj", name="pq00")
        q_proj_mms(ps00, 0, 0, 0, KT)
        ps01 = p_ps.tile([128, 512], F32, tag="proj", name="pq01")
        q_proj_mms(ps01, 0, 512, 0, KT)
        nc.vector.tensor_scalar(
            out=qT[:, 0, 0, 0:512], in0=ps00, scalar1=bq_sb[:, 0:1],
            scalar2=None, op0=ADD)
        nc.vector.tensor_scalar(
            out=qT[:, 0, 0, 512:1024], in0=ps01, scalar1=bq_sb[:, 0:1],
            scalar2=None, op0=ADD)
        k_proj_tile(0, 0, 256, pool=s_ps, tag="big")
        nc.vector.ln_bwd_dx(
            out=qT[:, 0, 1, 0:512], dy=ps00, x_hat=qT[:, 0, 0, 0:512],
            mean_dyx=1.0, mean_dy=bqn_sb[:, 0:1])
        nc.vector.ln_bwd_dx(
            out=qT[:, 0, 1, 512:1024], dy=ps01,
            x_hat=qT[:, 0, 0, 512:1024],
            mean_dyx=1.0, mean_dy=bqn_sb[:, 0:1])

        # ---- attention stream ----
        def scores_mms(h, lqh, lkt):
            ht, hp = h // 2, (h % 2) * 64
            ps = s_ps.tile([128, 1024], F32, tag="big",
                           name=f"s{h}_{lqh}_{lkt}")
            kap = kTt[hp:hp + 64, ht, lkt * 128:(lkt + 1) * 128]
            kap = kap.unsqueeze(1).broadcast_to([64, 2, 128])
            for n2 in range(2):
                c0 = lqh * 1024 + n2 * 512
                nc.tensor.matmul(
                    ps[:, n2 * 512:(n2 + 1) * 512],
                    kap, qT[hp:hp + 64, ht, :, c0:c0 + 512],
                    start=True, stop=True, perf_mode=DR)
            return ps

        def scores_exp(ps, h, lqh, lkt):
            et = e_pool.tile([128, 1024], BF16, tag="e",
                             name=f"e{h}_{lqh}_{lkt}")
            nc.scalar.activation(et, ps, AF.Exp, scale=0.125)
            return et

        def scores_tile_first():
            # tile (h0, lqh0, lkt0) from the hi plane only (regular
            # matmul): skips the lo-plane drain dependency so the first
            # exps fire ~2us earlier; the lo term is ~2% of q on 1/512
            # of the scores -> negligible.
            ps = s_ps.tile([128, 1024], F32, tag="big", name="s_first")
            et = e_pool.tile([128, 1024], BF16, tag="e", name="e_first")
            kap = kTt[0:64, 0, 0:128]
            for n2 in range(2):
                nc.tensor.matmul(
                    ps[:, n2 * 512:(n2 + 1) * 512], kap,
                    qT[0:64, 0, 0, n2 * 512:(n2 + 1) * 512],
                    start=True, stop=True)
                nc.scalar.activation(
                    et[:, n2 * 512:(n2 + 1) * 512],
                    ps[:, n2 * 512:(n2 + 1) * 512], AF.Exp, scale=0.125)
            return et

        norm_map = {}

        def attnv_chunk(h, lqh, c, e_tiles, alt=False):
            ht, g = h // 2, h % 2
            pool = s_ps if alt else av_ps
            o_t = pool.tile([128, 66], F32, tag="big" if alt else "av",
                            name=f"av{h}_{lqh}_{c}")
            for i, et in enumerate(e_tiles):
                nc.tensor.matmul(
                    o_t, et[:, c * 128:(c + 1) * 128],
                    v_sb[:, i, h * VSTR:h * VSTR + 66],
                    start=(i == 0), stop=(i == NLK - 1))
            rec = rec_pool.tile([128, 1], F32, tag="rec")
            nc.vector.reciprocal(out=rec, in_=o_t[:, 64:65])
            if g == 0:
                nst = norm_pool.tile([128, 128], BF16, tag="norm",
                                     name=f"n{ht}_{lqh}_{c}")
                norm_map[(lqh, ht, c)] = nst
            else:
                nst = norm_map.pop((lqh, ht, c))
            nc.vector.tensor_scalar(
                out=nst[:, g * 64:(g + 1) * 64], in0=o_t[:, 0:64],
                scalar1=rec, scalar2=None, op0=mybir.AluOpType.mult)
            if g == 1:
                c0 = lqh * 1024 + c * 128
                if lqh == 1 and ht == MT - 1:
                    # tail pair: PE transpose avoids DMA dispatch latency
                    tp = av_ps.tile([128, 128], BF16, tag="av",
                                    name=f"tp{c}")
                    for gg in range(2):
                        nc.tensor.matmul(
                            tp[64 * gg:64 * (gg + 1), :],
                            nst[:, gg * 64:(gg + 1) * 64], id_sb,
                            is_transpose=True, start=True, stop=True)
                    nc.vector.tensor_copy(
                        out=oT_all[:, ht, c0:c0 + 128], in_=tp)
                else:
                    nc.sync.dma_start_transpose(
                        out=oT_all[:, ht, c0:c0 + 128], in_=nst)

        e_saved = {}
        ps_next = [None]

        def next_mms(pos):
            # software pipeline: the score matmuls for pos land one
            # iteration early so their semaphores beat the exp stream
            if pos < 224:
                s2, t2 = pos // 16, pos % 16
                ps_next[0] = scores_mms(s2 % 8, s2 // 8, t2)

        for s in range(14):
            lqh, h = s // 8, s % 8
            pl, ph = (s - 1) // 8, (s - 1) % 8
            for t in range(16):
                pos = s * 16 + t
                if pos == 0:
                    et = scores_tile_first()
                    next_mms(1)
                else:
                    ps, ps_next[0] = ps_next[0], None
                    et = scores_exp(ps, h, lqh, t)
                    next_mms(pos + 1)
                e_saved.setdefault(s, []).append(et)
                if s == 1:
                    # v(hg0) lands ~pos 19: chunks doubled on tiles 5..8
                    # so the e-pool WAR on slot-1 tiles 8+ clears fast
                    in_chunks = 5 <= t < 9
                    if in_chunks:
                        attnv_chunk(ph, pl, 2 * (t - 5), e_saved[0])
                        attnv_chunk(ph, pl, 2 * (t - 5) + 1, e_saved[0])
                else:
                    in_chunks = s >= 1 and t < 8
                    if in_chunks:
                        attnv_chunk(ph, pl, t, e_saved[s - 1])
                pump(pos, 1 if in_chunks else 2)
            if s >= 1:
                e_saved.pop(s - 1)

        # all queue atoms (they reference x/w tiles) must be emitted
        # before the xw pool closes
        pump(10 ** 9, 10 ** 9)
        xw_ctx.close()

        # out-projection partial staging (pairs 0..2 + bo), bf16, all
        # in ONE tensor so the tail combines can read mt-pairs wide
        opart = ctx.enter_context(tc.tile_pool(name="opart", bufs=1))
        opart_t = opart.tile([128, 16, 512], BF16)
        bstage_pool = ctx.enter_context(tc.tile_pool(name="bstage", bufs=1))
        bstage = bstage_pool.tile([128, 2, KT, 512], BF16)
        part_sb = {}

        def opart_a(mt, n):
            c0 = n * 512
            ps = p_ps.tile([128, 512], F32, tag="proj", name=f"op{mt}n{n}")
            part_sb[(mt, n, "ps")] = ps
            for kt in (0, 1):
                nc.tensor.matmul(
                    ps, wo_sb[:, kt, mt * 128:(mt + 1) * 128],
                    oT_all[:, kt, c0:c0 + 512],
                    start=(kt == 0), stop=False)

        def opart_b(mt, n):
            c0 = n * 512
            ps = part_sb.pop((mt, n, "ps"))
            nc.tensor.matmul(
                ps, wo_sb[:, 2, mt * 128:(mt + 1) * 128],
                oT_all[:, 2, c0:c0 + 512], start=False, stop=True)
            nc.vector.tensor_scalar(
                out=opart_t[:, (n - 2) * KT + mt, :], in0=ps,
                scalar1=bo_sb[:, mt:mt + 1],
                scalar2=None, op0=ADD)

        pq = []
        for n in (2, 3):
            for mt in range(KT):
                pq.append(lambda mt=mt, n=n: opart_a(mt, n))
                pq.append(lambda mt=mt, n=n: opart_b(mt, n))
        pi = [0]

        def ppump(k):
            while k > 0 and pi[0] < len(pq):
                pq[pi[0]]()
                pi[0] += 1
                k -= 1

        # slots 14-15: lqh1 pair 3; out-proj partials pace in the gaps
        for s in (14, 15):
            lqh, h = s // 8, s % 8
            pl, ph = (s - 1) // 8, (s - 1) % 8
            for t in range(16):
                pos = s * 16 + t
                if ps_next[0] is None:
                    ps_next[0] = scores_mms(h, lqh, t)
                ps, ps_next[0] = ps_next[0], None
                et = scores_exp(ps, h, lqh, t)
                if pos + 1 < 256:
                    s2, t2 = (pos + 1) // 16, (pos + 1) % 16
                    ps_next[0] = scores_mms(s2 % 8, s2 // 8, t2)
                e_saved.setdefault(s, []).append(et)
                if t < 8:
                    attnv_chunk(ph, pl, t, e_saved[s - 1])
                    if s == 15:
                        ppump(1)
                else:
                    ppump(2)
            e_saved.pop(s - 1)

        # ---- tail: slot-15 attnv + pair-3 out-proj contribution ----
        ppump(100)
        lqh, h = 1, 7
        e_tiles = e_saved.pop(15)

        def oproj_final(j, n):
            # mt-pair final: the scores ring is idle in the tail, so
            # each pair gets a [128, 1024] psum tile, one wide DVE
            # tensor_tensor combine, and one batched 2-tile DMA
            c0 = n * 512
            ps = s_ps.tile([128, 1024], F32, tag="big", name=f"of{j}n{n}")
            for half in (0, 1):
                mt = 2 * j + half
                nc.tensor.matmul(
                    ps[:, half * 512:(half + 1) * 512],
                    wo_sb[:, 3, mt * 128:(mt + 1) * 128],
                    oT_all[:, 3, c0:c0 + 512], start=True, stop=True)
            idx = (n - 2) * KT + 2 * j
            nc.vector.tensor_tensor(
                out=bstage[:, n - 2, 2 * j:2 * j + 2, :], in0=ps,
                in1=opart_t[:, idx:idx + 2, :], op=ADD)
            dst = outT[2 * j * 128:(2 * j + 2) * 128,
                       c0:c0 + 512].rearrange("(q p) c -> p q c", p=128)
            nc.sync.dma_start(
                out=dst, in_=bstage[:, n - 2, 2 * j:2 * j + 2, :])

        for c in range(8):
            attnv_chunk(h, lqh, c, e_tiles, alt=(c % 2 == 1))
            if c in (5, 7):
                oproj_final(c - 5, 2)
                oproj_final(c - 4, 2)
        for j in range(KT // 2):
            oproj_final(j, 3)
